# revision 24
# baseline (speedup 1.0000x reference)
"""Trainium2 Bass kernel for BertSimSelfAttention (sparse_attention).

Problem (full): B=4, M=64, SEQ=256, DIM=1024, H=16, HD=64.
Effective batch rows R = B*SEQ = 1024, each row: m=64 tokens of dim=1024.
  hs  = transpose(hidden_states,(0,2,1,3)).reshape(R, 64, 1024)
  q/k/v = hs @ W{q,k,v}.T + b   (per token)
  per (row, head): scores = (q @ k.T)/8 * sim[row] + (-1e4)*(1-am[row,j])
  probs = softmax_j(scores);  ctx = probs @ v  -> out [R, 64, 1024]

Sharding: data-parallel over rows, 128 rows/core x 8 cores. The host
pre-transposes x, W (and sim per row) and converts them to bf16 so the
device consumes contraction-major bf16 layouts directly; the 1/sqrt(hd)
scale is folded into Wq/bq on the host.

Per-core design (~0.84 ms traced / ~0.70 ms untraced on TRN2):
  - Everything matmul-side is bf16 (fp32 PSUM accumulate). bf16
    128-col LDWEIGHTS (~107ns, FWL) hides under the 512-col moving
    stream (213ns); fp32r stationaries would gate the pipe at ~273ns.
  - Projections: q/k och-groups emit q- and k-streams interleaved
    across two PSUM banks; v reuses its xt stationary for both output
    halves. q/k bias added by ACT Identity at evacuation; v evacuated
    on DVE; v bias accumulated as a K=1 matmul (skipped when bv == 0).
  - Attention runs per ROW-PAIR on 64x64 PE tiles with every tile
    owning a unique (PSUM bank, partition half) -- concurrent PE row
    groups must never write the same PSUM partitions:
      scores  S'[j,q]: bank A (even heads, contraction strip 0)
        T0=even row [0:64], T2=odd row [64:128]; bank B (odd heads,
        strip 64): T8=even [0:64], T10=odd [64:128].
      ctx: T0=ctxA[0:64] (even), T10=ctxA[64:128] (odd),
        T8=ctxB[0:64] (odd), T2=ctxB[64:128] (even).
    The 4-way tile spread runs scores/ctx matmuls concurrently
    (~15ns/MM vs 27ns serial).
  - softmax over j (= partitions): t = S' * simT (DVE, sim bf16 via
    stride-0 broadcast); e = exp(t + maskcol) on ACT with a
    per-partition row-pair bias column (masked lanes give exact 0);
    denominators via one PE matmul per bank with a block-ones
    stationary; reciprocal_approx_fast in-place on PSUM;
    probs.T = e * recip (bf16) feeds ctx directly as stationary.
  - v's natural [t, o] layout feeds ctx moving operands with no
    duplication; output DMAs are contiguous [64, 512] bf16 lines in a
    parity-blocked column layout that the host permutes back.
  - Emission software-pipelines tile i's 12 projection groups with
    tile (i-1)'s attention pair units (one-pair lookahead).
"""

import sys

sys.path.insert(0, "/opt/trn_rl_repo")

import numpy as np
import concourse.bass as bass
import concourse.bacc as bacc
import concourse.mybir as mybir
import concourse.tile as tile

F32 = mybir.dt.float32
F32R = mybir.dt.float32r
BF16 = mybir.dt.bfloat16
AF = mybir.ActivationFunctionType
ALU = mybir.AluOpType

N_CORES = 8
M = 64                    # tokens per row
DIM = 1024
H = 16
HD = 64
NEG = -10000.0


def build_core_kernel(nc, n_tiles=16, rows_per_tile=8, debug=False, use_bv=True):
    """Emit the per-core program. tile = rows_per_tile rows (must be even)."""
    T_TILE = rows_per_tile * M        # tokens per tile (512 default)
    n_rows = n_tiles * rows_per_tile
    n_tok = n_rows * M
    SUB = T_TILE // 128               # 128-token subtiles per tile

    xt_d = nc.dram_tensor("xT", (DIM, n_tok), BF16, kind="ExternalInput")
    sim_d = nc.dram_tensor("simg", (n_rows, M, M), BF16, kind="ExternalInput")
    am_d = nc.dram_tensor("am", (n_rows, M), F32, kind="ExternalInput")
    wq_d = nc.dram_tensor("WqT", (DIM, DIM), BF16, kind="ExternalInput")
    wk_d = nc.dram_tensor("WkT", (DIM, DIM), BF16, kind="ExternalInput")
    wv_d = nc.dram_tensor("WvT", (DIM, DIM), BF16, kind="ExternalInput")
    bq_d = nc.dram_tensor("bq", (DIM,), F32, kind="ExternalInput")
    bk_d = nc.dram_tensor("bk", (DIM,), F32, kind="ExternalInput")
    bv_d = nc.dram_tensor("bv", (DIM,), F32, kind="ExternalInput")
    id_d = nc.dram_tensor("ident", (128, 128), F32, kind="ExternalInput")
    sel_d = nc.dram_tensor("selm", (128, 2), F32, kind="ExternalInput")
    bsel_d = nc.dram_tensor("bselm", (2, 128), F32, kind="ExternalInput")
    out_d = nc.dram_tensor("out", (n_tok, DIM), BF16, kind="ExternalOutput")

    dbg = {}
    if debug:
        dbg["qt"] = nc.dram_tensor("dbg_qt", (DIM, n_tok), F32, kind="ExternalOutput")
        dbg["kt"] = nc.dram_tensor("dbg_kt", (DIM, n_tok), F32, kind="ExternalOutput")
        dbg["v"] = nc.dram_tensor("dbg_v", (n_tok, DIM), F32, kind="ExternalOutput")
        dbg["pr"] = nc.dram_tensor("dbg_pr", (n_rows, 128, 512), F32,
                                   kind="ExternalOutput")
        dbg["s"] = nc.dram_tensor("dbg_s", (n_rows, 128, 512), F32,
                                  kind="ExternalOutput")

    with tile.TileContext(nc) as tc:
        with (
            tc.tile_pool(name="consts", bufs=1) as consts,
            tc.tile_pool(name="stage", bufs=3) as stage,
            tc.tile_pool(name="xtp", bufs=2) as xtp,
            tc.tile_pool(name="qkp", bufs=2) as qkp,
            tc.tile_pool(name="vp", bufs=2) as vp,
            tc.tile_pool(name="rowp", bufs=2) as rowp,
            tc.tile_pool(name="small_ps", bufs=2, space="PSUM") as small_ps,
            tc.tile_pool(name="proj_ps", bufs=2, space="PSUM") as proj_ps,
            tc.tile_pool(name="att_ps", bufs=4, space="PSUM") as att_ps,
        ):
            # ---------------- tiny consts first ----------------
            ident = consts.tile([128, 128], F32)
            nc.sync.dma_start(ident[:], id_d[:])

            am_all = consts.tile([128, M], F32)
            if n_rows < 128:
                nc.gpsimd.memset(am_all[:], 1.0)
            nc.sync.dma_start(am_all[0:n_rows, :], am_d[:])

            # block-ones selector: halfones[p, m] = 1 iff same 64-half;
            # halfones.T @ e sums each half and broadcasts to its partitions
            halfones = consts.tile([128, 128], BF16)
            nc.gpsimd.memset(halfones[:], 0.0)
            nc.gpsimd.memset(halfones[0:64, 0:64], 1.0)
            nc.gpsimd.memset(halfones[64:128, 64:128], 1.0)

            bq_sb = consts.tile([128, 8], F32)
            bk_sb = consts.tile([128, 8], F32)
            nc.sync.dma_start(bq_sb[:], bq_d[:].rearrange("(o p) -> p o", p=128))
            nc.sync.dma_start(bk_sb[:], bk_d[:].rearrange("(o p) -> p o", p=128))

            if use_bv:
                # bv as a K=1 bf16 pair for psum-accumulate
                ones_f = consts.tile([1, 128], F32)
                nc.gpsimd.memset(ones_f[:], 1.0)
                ones_r = consts.tile([1, 128], BF16)
                nc.vector.tensor_copy(ones_r[:], ones_f[:])
                bv_row = consts.tile([1, DIM], F32)
                nc.sync.dma_start(bv_row[:],
                                  bv_d[:].rearrange("(a o) -> a o", a=1))
                bv_r = consts.tile([1, DIM], BF16)
                nc.vector.tensor_copy(bv_r[:], bv_row[:])

            # mask bias columns, row-pair layout: mcol[64*(r%2)+j, r//2] =
            # -1e4*(1 - am[r, j])  (exp-bias per key token j, even row on
            # strip 0, odd row on strip 1)
            n_pairs = n_rows // 2
            mcol = consts.tile([128, n_pairs], F32)
            modd = consts.tile([64, n_pairs], F32)
            amt_ps = small_ps.tile([128, 128], F32, tag="srb")
            nc.tensor.transpose(amt_ps[0:M, 0:128], am_all[:], ident[:])
            amt_v = amt_ps[0:M, 0:2 * n_pairs].rearrange(
                "j (p two) -> j p two", two=2)
            nc.vector.tensor_scalar(
                mcol[0:64, :].rearrange("j (p a) -> j p a", a=1),
                amt_v[:, :, 0:1], -NEG, NEG,
                op0=ALU.mult, op1=ALU.add)
            nc.vector.tensor_scalar(
                modd[:].rearrange("j (p a) -> j p a", a=1),
                amt_v[:, :, 1:2], -NEG, NEG,
                op0=ALU.mult, op1=ALU.add)
            nc.gpsimd.dma_start(mcol[64:128, :], modd[:])

            # ---------------- weights (+ tile-0 x interleaved) ----------
            def emit_xt(ti):
                t0 = ti * T_TILE
                xt = [xtp.tile([128, T_TILE], BF16, tag=f"xt{d}",
                               name=f"xt{d}_{ti}") for d in range(8)]
                for dch in range(8):
                    nc.sync.dma_start(
                        xt[dch][:],
                        xt_d[128 * dch:128 * dch + 128, t0:t0 + T_TILE])
                return xt

            dma_engs = [nc.sync, nc.gpsimd, nc.scalar]

            def emit_w(name, w_d, dchs):
                wt = wts[name]
                for dch in dchs:
                    weng = dma_engs[dch % 2 if name != "q" else dch % 3]
                    weng.dma_start(
                        wt[dch][:], w_d[128 * dch:128 * dch + 128, :])

            wts = {name: [consts.tile([128, DIM], BF16, tag=f"w{name}{d}",
                                      name=f"w{name}{d}") for d in range(8)]
                   for name in ("q", "k", "v")}
            xt0 = [xtp.tile([128, T_TILE], BF16, tag=f"xt{d}",
                             name=f"xt{d}_0") for d in range(8)]
            for dch in range(8):
                emit_w("q", wq_d, [dch])
                dma_engs[(dch + 1) % 3].dma_start(
                    xt0[dch][:], xt_d[128 * dch:128 * dch + 128, 0:T_TILE])
            emit_w("k", wk_d, range(8))
            emit_w("v", wv_d, range(8))
            wqt, wkt, wvt = wts["q"], wts["k"], wts["v"]

            # ---------------- main loop over token tiles ----------------
            # Emission interleaves tile ti's projection groups with tile
            # (ti-1)'s attention rows so the PE program order has dense
            # matmul work to fill softmax dependency stalls (keeps HAM warm).

            def make_proj(ti, xt):
                qt = [qkp.tile([128, T_TILE], BF16, tag=f"qt{o}",
                               name=f"qt{o}_{ti}") for o in range(8)]
                kt = [qkp.tile([128, T_TILE], BF16, tag=f"kt{o}",
                               name=f"kt{o}_{ti}") for o in range(8)]
                vts = [vp.tile([128, DIM], BF16, tag=f"v{s}",
                               name=f"v{s}_{ti}") for s in range(SUB)]
                groups = []

                def qk_group(och):
                    # q and k streams interleaved across two PSUM banks so
                    # consecutive matmuls never target the same bank region
                    ps_q = proj_ps.tile([128, T_TILE], F32, tag="proj",
                                        name=f"qps{och}_{ti}")
                    ps_k = proj_ps.tile([128, T_TILE], F32, tag="proj",
                                        name=f"kps{och}_{ti}")
                    osl = slice(128 * och, 128 * och + 128)
                    for dch in range(8):
                        nc.tensor.matmul(
                            ps_q[:], wqt[dch][:, osl], xt[dch][:],
                            start=(dch == 0), stop=(dch == 7),
                            skip_group_check=True)
                        nc.tensor.matmul(
                            ps_k[:], wkt[dch][:, osl], xt[dch][:],
                            start=(dch == 0), stop=(dch == 7),
                            skip_group_check=True)
                    nc.scalar.activation(
                        qt[och][:], ps_q[:], AF.Identity,
                        bias=bq_sb[:, och:och + 1], scale=1.0)
                    nc.scalar.activation(
                        kt[och][:], ps_k[:], AF.Identity,
                        bias=bk_sb[:, och:och + 1], scale=1.0)

                def v_group(sub):
                    # both output halves share the stationary xt strip
                    vt = vts[sub]
                    ps = [proj_ps.tile([128, 512], F32, tag="proj",
                                       name=f"vps{sub}{oh}_{ti}")
                          for oh in range(2)]
                    xsl = slice(128 * sub, 128 * sub + 128)
                    for dch in range(8):
                        for oh in range(2):
                            nc.tensor.matmul(
                                ps[oh][:], xt[dch][:, xsl],
                                wvt[dch][:, 512 * oh:512 * oh + 512],
                                start=(dch == 0),
                                stop=(dch == 7) and not use_bv,
                                skip_group_check=True)
                    for oh in range(2):
                        sl = slice(512 * oh, 512 * oh + 512)
                        if use_bv:
                            nc.tensor.matmul(
                                ps[oh][:], ones_r[:], bv_r[:, sl],
                                start=False, stop=True,
                                skip_group_check=True)
                        nc.vector.tensor_copy(vt[:, sl], ps[oh][:])

                for och in range(8):
                    groups.append(lambda och=och: qk_group(och))
                for sub in range(SUB):
                    groups.append(lambda sub=sub: v_group(sub))
                return qt, kt, vts, groups

            def make_att_rows(ti, qt, kt, vts, la=1):
                # Row-pair units: rows (2pi, 2pi+1) live on partition strips
                # (0, 64) of shared PSUM banks; all 16 heads of a row stay on
                # its strip, so v's natural layout feeds ctx directly (no
                # duplication) and the output DMA is fully contiguous.
                pairs = rows_per_tile // 2
                pairstate = {}

                def att_pair_a(pi):
                    pr = ti * pairs + pi          # global pair index

                    simt2 = rowp.tile([128, M], BF16, tag="sim2",
                                      name=f"sim2_{pr}")
                    nc.gpsimd.dma_start(simt2[0:64, :], sim_d[2 * pr, :, :])
                    nc.gpsimd.dma_start(simt2[64:128, :],
                                        sim_d[2 * pr + 1, :, :])

                    # scores transposed: S'[j, q]; bank bk holds heads of
                    # parity bk (h = 2hh+bk), contraction strip fixed at
                    # 64*bk, so each PE tile owns a unique (bank, half):
                    #   A[0:64]=T0  A[64:128]=T2  B[0:64]=T8  B[64:128]=T10
                    # (concurrent row-groups never share psum partitions)
                    tsl_e = slice(M * 2 * pi, M * 2 * pi + M)
                    tsl_o = slice(M * (2 * pi + 1), M * (2 * pi + 1) + M)
                    banks = [att_ps.tile([128, 512], F32, tag="att",
                                         name=f"s_{pr}_{bk}")
                             for bk in range(2)]
                    for hh in range(8):
                        for bk in range(2):
                            st = 64 * bk
                            for strip, tsl in ((0, tsl_e), (64, tsl_o)):
                                nc.tensor.matmul(
                                    banks[bk][strip:strip + 64,
                                              64 * hh:64 * hh + 64],
                                    kt[hh][st:st + 64, tsl],
                                    qt[hh][st:st + 64, tsl],
                                    start=True, stop=True,
                                    tile_position=(st, strip),
                                )

                    pts = []
                    for bk in range(2):
                        s_ps = banks[bk]
                        # t = S' * simT;  e = exp(t + maskcol)  (bf16 out)
                        tt = rowp.tile([128, 512], F32, tag=f"tt{bk}",
                                       name=f"tt_{pr}_{bk}")
                        nc.vector.tensor_tensor(
                            tt[:].rearrange("p (a j) -> p a j", j=M),
                            s_ps[:].rearrange("p (a j) -> p a j", j=M),
                            simt2[:].rearrange("p (a j) -> p a j", a=1)
                            .broadcast_to([128, 8, M]),
                            op=ALU.mult,
                        )
                        et = rowp.tile([128, 512], BF16, tag=f"et{bk}",
                                       name=f"et_{pr}_{bk}")
                        nc.scalar.activation(et[:], tt[:], AF.Exp,
                                             bias=mcol[:, pr:pr + 1])

                        # denominators summed + broadcast in one PE matmul
                        dn_ps = small_ps.tile([128, 512], F32, tag="srb",
                                              name=f"dn_{pr}_{bk}")
                        nc.tensor.matmul(dn_ps[:], halfones[:], et[:],
                                         start=True, stop=True)
                        nc.vector.reciprocal_approx_fast(out=dn_ps[:],
                                                         in_=dn_ps[:])
                        pt = rowp.tile([128, 512], BF16, tag=f"pt{bk}",
                                       name=f"pt_{pr}_{bk}")
                        nc.vector.tensor_tensor(pt[:], et[:], dn_ps[:],
                                                op=ALU.mult)
                        pts.append(pt)
                    pairstate[pi] = pts

                def att_pair_b(pi):
                    pr = ti * pairs + pi
                    pts = pairstate.pop(pi)
                    vt = vts[pi]   # [128 tok (both rows), DIM] natural

                    # ctx tile ownership (unique bank+half per PE tile):
                    #   ctxA[0:64]=T0(even)  ctxA[64:128]=T10(odd)
                    #   ctxB[0:64]=T8(odd)   ctxB[64:128]=T2(even)
                    ctxs = [att_ps.tile([128, 512], F32, tag="att",
                                        name=f"ctx_{pr}_{bk}")
                            for bk in range(2)]
                    for hh in range(8):
                        for bk in range(2):
                            h = 2 * hh + bk
                            for js, outs in (((0, 0) if bk == 0 else (0, 64)),
                                             ((64, 64) if bk == 0
                                              else (64, 0)),):
                                nc.tensor.matmul(
                                    ctxs[bk][outs:outs + 64,
                                             64 * hh:64 * hh + 64],
                                    pts[bk][js:js + 64,
                                            64 * hh:64 * hh + 64],
                                    vt[js:js + 64, 64 * h:64 * h + 64],
                                    start=True, stop=True,
                                    tile_position=(js, outs),
                                )
                    # osb_A: [0:64]=even row, [64:128]=odd row (even heads)
                    # osb_B: [0:64]=odd row,  [64:128]=even row (odd heads)
                    # out_dev columns parity-blocked; host permutes back
                    for bk in range(2):
                        osb = rowp.tile([128, 512], BF16, tag=f"osb{bk}",
                                        name=f"osb_{pr}_{bk}", bufs=2)
                        nc.vector.tensor_copy(osb[:], ctxs[bk][:])
                        for sp in range(2):
                            r = 2 * pr + (sp if bk == 0 else 1 - sp)
                            nc.sync.dma_start(
                                out_d[M * r:M * r + M,
                                      512 * bk:512 * bk + 512],
                                osb[64 * sp:64 * sp + 64, :])

                # lookahead pipeline (la pairs of softmax chain in flight)
                units = [lambda pi=pi: att_pair_a(pi) for pi in range(la)]
                for pi in range(la, pairs):
                    units.append(lambda pi=pi: att_pair_a(pi))
                    units.append(lambda pi=pi: att_pair_b(pi - la))
                for pi in range(pairs - la, pairs):
                    units.append(lambda pi=pi: att_pair_b(pi))
                return units

            prev_rows = []
            for ti in range(n_tiles):
                xt = xt0 if ti == 0 else emit_xt(ti)
                qt, kt, vts, groups = make_proj(ti, xt)
                ri = 0
                for gi, g in enumerate(groups):
                    g()
                    while (ri < len(prev_rows)
                           and (gi + 1) * len(prev_rows) // len(groups) > ri):
                        prev_rows[ri]()
                        ri += 1
                while ri < len(prev_rows):
                    prev_rows[ri]()
                    ri += 1
                prev_rows = make_att_rows(
                    ti, qt, kt, vts, la=(2 if ti == n_tiles - 1 else 1))
            for row in prev_rows:
                row()

    return dict(out=out_d)


def _prepare_shards(hidden_states, attention_mask, sim_graph, Wq, bq, Wk, bk, Wv, bv,
                    n_cores=N_CORES):
    b, m, seq, dim = hidden_states.shape
    R = b * seq
    hs = np.transpose(np.asarray(hidden_states), (0, 2, 1, 3)).reshape(R, m, dim)
    import ml_dtypes
    bf16 = ml_dtypes.bfloat16
    am = np.ascontiguousarray(
        np.transpose(np.asarray(attention_mask), (0, 2, 1)).reshape(R, m),
        dtype=np.float32)
    sim = np.ascontiguousarray(
        np.transpose(np.asarray(sim_graph), (0, 2, 1)).astype(bf16))
    ident = np.eye(128, dtype=np.float32)
    selm = np.zeros((128, 2), np.float32)
    selm[0:64, 0] = 1.0
    selm[64:128, 1] = 1.0
    bselm = np.zeros((2, 128), np.float32)
    bselm[0, 0:64] = 1.0
    bselm[1, 64:128] = 1.0
    WqT = np.ascontiguousarray((np.asarray(Wq).T * 0.125).astype(bf16))
    WkT = np.ascontiguousarray(np.asarray(Wk).T.astype(bf16))
    WvT = np.ascontiguousarray(np.asarray(Wv).T.astype(bf16))
    rows_per_core = R // n_cores
    in_maps = []
    for c in range(n_cores):
        r0 = c * rows_per_core
        xT = np.ascontiguousarray(
            hs[r0:r0 + rows_per_core].reshape(rows_per_core * m, dim).T
            .astype(bf16))
        in_maps.append(dict(
            xT=xT,
            simg=sim[r0:r0 + rows_per_core],
            am=am[r0:r0 + rows_per_core],
            WqT=WqT, WkT=WkT, WvT=WvT,
            bq=np.ascontiguousarray(np.asarray(bq) * 0.125, np.float32),
            bk=np.ascontiguousarray(bk, np.float32),
            bv=np.ascontiguousarray(bv, np.float32),
            ident=ident, selm=selm, bselm=bselm,
        ))
    return in_maps


_CACHE = {}


def _get_compiled(use_bv=True):
    key = ("nc", use_bv)
    if key not in _CACHE:
        nc = bacc.Bacc("TRN2", target_bir_lowering=False, debug=False)
        build_core_kernel(nc, use_bv=use_bv)
        nc.compile()
        _CACHE[key] = nc
    return _CACHE[key]


LAST_EXEC_NS = [None]
LAST_RESULT = [None]


def kernel(hidden_states, attention_mask, sim_graph, Wq, bq, Wk, bk, Wv, bv,
           b=4, m=64, seq=256, dim=1024, **_):
    import os
    from concourse.bass_utils import run_bass_kernel_spmd

    use_bv = bool(np.any(np.asarray(bv)))
    nc = _get_compiled(use_bv=use_bv)
    in_maps = _prepare_shards(hidden_states, attention_mask, sim_graph,
                              Wq, bq, Wk, bk, Wv, bv)
    trace = bool(int(os.environ.get("BERT_TRACE", "0")))
    if trace:
        try:  # register the NTFF hook if the middleware didn't
            from antenv.axon_hooks import (get_axon_ntff_profile_hook,
                                           set_axon_ntff_profile_hook)
            if get_axon_ntff_profile_hook() is None:
                from trn_agent_boot.trn_boot import _ntff_profile_via_ctypes
                set_axon_ntff_profile_hook(
                    _ntff_profile_via_ctypes("/opt/axon/libaxon_pjrt.so"))
        except Exception:
            trace = False
    res = run_bass_kernel_spmd(nc, in_maps, list(range(N_CORES)), trace=trace)
    LAST_EXEC_NS[0] = res.exec_time_ns
    LAST_RESULT[0] = res
    R = int(b) * int(seq)
    out = np.concatenate([np.asarray(res.results[c]["out"], dtype=np.float32)
                          for c in range(N_CORES)], axis=0)
    # device column layout is parity-blocked: col = 512*(h%2) + 64*(h//2) + hd
    out = out.reshape(-1, 2, 8, 64).transpose(0, 2, 1, 3)
    return np.ascontiguousarray(out).reshape(R, int(m), int(dim))



# revision 27
# speedup vs baseline: 1.0273x; 1.0273x over previous
"""Trainium2 Bass kernel for BertSimSelfAttention (sparse_attention).

Problem (full): B=4, M=64, SEQ=256, DIM=1024, H=16, HD=64.
Effective batch rows R = B*SEQ = 1024, each row: m=64 tokens of dim=1024.
  hs  = transpose(hidden_states,(0,2,1,3)).reshape(R, 64, 1024)
  q/k/v = hs @ W{q,k,v}.T + b   (per token)
  per (row, head): scores = (q @ k.T)/8 * sim[row] + (-1e4)*(1-am[row,j])
  probs = softmax_j(scores);  ctx = probs @ v  -> out [R, 64, 1024]

Sharding: data-parallel over rows, 128 rows/core x 8 cores. The host
pre-transposes x, W (and sim per row) and converts them to bf16 so the
device consumes contraction-major bf16 layouts directly; the 1/sqrt(hd)
scale is folded into Wq/bq on the host.

Per-core design (~0.84 ms traced / ~0.70 ms untraced on TRN2):
  - Everything matmul-side is bf16 (fp32 PSUM accumulate). bf16
    128-col LDWEIGHTS (~107ns, FWL) hides under the 512-col moving
    stream (213ns); fp32r stationaries would gate the pipe at ~273ns.
  - Projections: q/k och-groups emit q- and k-streams interleaved
    across two PSUM banks; v reuses its xt stationary for both output
    halves. q/k bias added by ACT Identity at evacuation; v evacuated
    on DVE; v bias accumulated as a K=1 matmul (skipped when bv == 0).
  - Attention runs per ROW-PAIR on 64x64 PE tiles with every tile
    owning a unique (PSUM bank, partition half) -- concurrent PE row
    groups must never write the same PSUM partitions:
      scores  S'[j,q]: bank A (even heads, contraction strip 0)
        T0=even row [0:64], T2=odd row [64:128]; bank B (odd heads,
        strip 64): T8=even [0:64], T10=odd [64:128].
      ctx: T0=ctxA[0:64] (even), T10=ctxA[64:128] (odd),
        T8=ctxB[0:64] (odd), T2=ctxB[64:128] (even).
    The 4-way tile spread runs scores/ctx matmuls concurrently
    (~15ns/MM vs 27ns serial).
  - softmax over j (= partitions): t = S' * simT (DVE, sim bf16 via
    stride-0 broadcast); e = exp(t + maskcol) on ACT with a
    per-partition row-pair bias column (masked lanes give exact 0);
    denominators via one PE matmul per bank with a block-ones
    stationary; reciprocal_approx_fast in-place on PSUM;
    probs.T = e * recip (bf16) feeds ctx directly as stationary.
  - v's natural [t, o] layout feeds ctx moving operands with no
    duplication; output DMAs are contiguous [64, 512] bf16 lines in a
    parity-blocked column layout that the host permutes back.
  - Emission software-pipelines tile i's 12 projection groups with
    tile (i-1)'s attention pair units (one-pair lookahead).
"""

import sys

sys.path.insert(0, "/opt/trn_rl_repo")

import numpy as np
import concourse.bass as bass
import concourse.bacc as bacc
import concourse.mybir as mybir
import concourse.tile as tile

F32 = mybir.dt.float32
F32R = mybir.dt.float32r
BF16 = mybir.dt.bfloat16
AF = mybir.ActivationFunctionType
ALU = mybir.AluOpType

N_CORES = 8
M = 64                    # tokens per row
DIM = 1024
H = 16
HD = 64
NEG = -10000.0


def build_core_kernel(nc, n_tiles=16, rows_per_tile=8, debug=False, use_bv=True):
    """Emit the per-core program. tile = rows_per_tile rows (must be even)."""
    T_TILE = rows_per_tile * M        # tokens per tile (512 default)
    n_rows = n_tiles * rows_per_tile
    n_tok = n_rows * M
    SUB = T_TILE // 128               # 128-token subtiles per tile

    xt_d = nc.dram_tensor("xT", (DIM, n_tok), BF16, kind="ExternalInput")
    sim_d = nc.dram_tensor("simg", (n_rows, M, M), BF16, kind="ExternalInput")
    am_d = nc.dram_tensor("am", (n_rows, M), F32, kind="ExternalInput")
    wq_d = nc.dram_tensor("WqT", (DIM, DIM), BF16, kind="ExternalInput")
    wk_d = nc.dram_tensor("WkT", (DIM, DIM), BF16, kind="ExternalInput")
    wv_d = nc.dram_tensor("WvT", (DIM, DIM), BF16, kind="ExternalInput")
    bq_d = nc.dram_tensor("bq", (DIM,), F32, kind="ExternalInput")
    bk_d = nc.dram_tensor("bk", (DIM,), F32, kind="ExternalInput")
    bv_d = nc.dram_tensor("bv", (DIM,), F32, kind="ExternalInput")
    id_d = nc.dram_tensor("ident", (128, 128), F32, kind="ExternalInput")
    sel_d = nc.dram_tensor("selm", (128, 2), F32, kind="ExternalInput")
    bsel_d = nc.dram_tensor("bselm", (2, 128), F32, kind="ExternalInput")
    out_d = nc.dram_tensor("out", (n_tok, DIM), BF16, kind="ExternalOutput")

    dbg = {}
    if debug:
        dbg["qt"] = nc.dram_tensor("dbg_qt", (DIM, n_tok), F32, kind="ExternalOutput")
        dbg["kt"] = nc.dram_tensor("dbg_kt", (DIM, n_tok), F32, kind="ExternalOutput")
        dbg["v"] = nc.dram_tensor("dbg_v", (n_tok, DIM), F32, kind="ExternalOutput")
        dbg["pr"] = nc.dram_tensor("dbg_pr", (n_rows, 128, 512), F32,
                                   kind="ExternalOutput")
        dbg["s"] = nc.dram_tensor("dbg_s", (n_rows, 128, 512), F32,
                                  kind="ExternalOutput")

    with tile.TileContext(nc) as tc:
        with (
            tc.tile_pool(name="consts", bufs=1) as consts,
            tc.tile_pool(name="stage", bufs=3) as stage,
            tc.tile_pool(name="xtp", bufs=2) as xtp,
            tc.tile_pool(name="qkp", bufs=2) as qkp,
            tc.tile_pool(name="vp", bufs=2) as vp,
            tc.tile_pool(name="rowp", bufs=2) as rowp,
            tc.tile_pool(name="small_ps", bufs=2, space="PSUM") as small_ps,
            tc.tile_pool(name="proj_ps", bufs=2, space="PSUM") as proj_ps,
            tc.tile_pool(name="att_ps", bufs=4, space="PSUM") as att_ps,
        ):
            # ---------------- tiny consts first ----------------
            ident = consts.tile([128, 128], F32)
            nc.sync.dma_start(ident[:], id_d[:])

            am_all = consts.tile([128, M], F32)
            if n_rows < 128:
                nc.gpsimd.memset(am_all[:], 1.0)
            nc.sync.dma_start(am_all[0:n_rows, :], am_d[:])

            # block-ones selector: halfones[p, m] = 1 iff same 64-half;
            # halfones.T @ e sums each half and broadcasts to its partitions
            halfones = consts.tile([128, 128], BF16)
            nc.gpsimd.memset(halfones[:], 0.0)
            nc.gpsimd.memset(halfones[0:64, 0:64], 1.0)
            nc.gpsimd.memset(halfones[64:128, 64:128], 1.0)

            bq_sb = consts.tile([128, 8], F32)
            bk_sb = consts.tile([128, 8], F32)
            nc.sync.dma_start(bq_sb[:], bq_d[:].rearrange("(o p) -> p o", p=128))
            nc.sync.dma_start(bk_sb[:], bk_d[:].rearrange("(o p) -> p o", p=128))

            if use_bv:
                # bv as a K=1 bf16 pair for psum-accumulate
                ones_f = consts.tile([1, 128], F32)
                nc.gpsimd.memset(ones_f[:], 1.0)
                ones_r = consts.tile([1, 128], BF16)
                nc.vector.tensor_copy(ones_r[:], ones_f[:])
                bv_row = consts.tile([1, DIM], F32)
                nc.sync.dma_start(bv_row[:],
                                  bv_d[:].rearrange("(a o) -> a o", a=1))
                bv_r = consts.tile([1, DIM], BF16)
                nc.vector.tensor_copy(bv_r[:], bv_row[:])

            # mask bias columns, row-pair layout: mcol[64*(r%2)+j, r//2] =
            # -1e4*(1 - am[r, j])  (exp-bias per key token j, even row on
            # strip 0, odd row on strip 1)
            n_pairs = n_rows // 2
            mcol = consts.tile([128, n_pairs], F32)
            modd = consts.tile([64, n_pairs], F32)
            amt_ps = small_ps.tile([128, 128], F32, tag="srb")
            nc.tensor.transpose(amt_ps[0:M, 0:128], am_all[:], ident[:])
            amt_v = amt_ps[0:M, 0:2 * n_pairs].rearrange(
                "j (p two) -> j p two", two=2)
            nc.vector.tensor_scalar(
                mcol[0:64, :].rearrange("j (p a) -> j p a", a=1),
                amt_v[:, :, 0:1], -NEG, NEG,
                op0=ALU.mult, op1=ALU.add)
            nc.vector.tensor_scalar(
                modd[:].rearrange("j (p a) -> j p a", a=1),
                amt_v[:, :, 1:2], -NEG, NEG,
                op0=ALU.mult, op1=ALU.add)
            nc.gpsimd.dma_start(mcol[64:128, :], modd[:])

            # ---------------- weights (+ tile-0 x interleaved) ----------
            def emit_xt(ti):
                t0 = ti * T_TILE
                xt = [xtp.tile([128, T_TILE], BF16, tag=f"xt{d}",
                               name=f"xt{d}_{ti}") for d in range(8)]
                for dch in range(8):
                    nc.sync.dma_start(
                        xt[dch][:],
                        xt_d[128 * dch:128 * dch + 128, t0:t0 + T_TILE])
                return xt

            dma_engs = [nc.sync, nc.gpsimd, nc.scalar]

            def emit_w(name, w_d, dchs):
                wt = wts[name]
                for dch in dchs:
                    weng = dma_engs[dch % 2 if name != "q" else dch % 3]
                    weng.dma_start(
                        wt[dch][:], w_d[128 * dch:128 * dch + 128, :])

            wts = {name: [consts.tile([128, DIM], BF16, tag=f"w{name}{d}",
                                      name=f"w{name}{d}") for d in range(8)]
                   for name in ("q", "k", "v")}
            xt0 = [xtp.tile([128, T_TILE], BF16, tag=f"xt{d}",
                             name=f"xt{d}_0") for d in range(8)]
            for dch in range(8):
                emit_w("q", wq_d, [dch])
                dma_engs[(dch + 1) % 3].dma_start(
                    xt0[dch][:], xt_d[128 * dch:128 * dch + 128, 0:T_TILE])
            emit_w("k", wk_d, range(8))
            emit_w("v", wv_d, range(8))
            wqt, wkt, wvt = wts["q"], wts["k"], wts["v"]

            # ---------------- main loop over token tiles ----------------
            # Emission interleaves tile ti's projection groups with tile
            # (ti-1)'s attention rows so the PE program order has dense
            # matmul work to fill softmax dependency stalls (keeps HAM warm).

            def make_proj(ti, xt):
                qt = [qkp.tile([128, T_TILE], BF16, tag=f"qt{o}",
                               name=f"qt{o}_{ti}") for o in range(8)]
                kt = [qkp.tile([128, T_TILE], BF16, tag=f"kt{o}",
                               name=f"kt{o}_{ti}") for o in range(8)]
                vts = [vp.tile([128, DIM], BF16, tag=f"v{s}",
                               name=f"v{s}_{ti}") for s in range(SUB)]
                groups = []

                def qk_group(wt, dst, b_sb, och):
                    # one bank, one FWL weight-load per 512-col stream --
                    # two interleaved streams saturate the LD path (XBUS)
                    ps = proj_ps.tile([128, T_TILE], F32, tag="proj",
                                      name=f"qkps{och}_{ti}")
                    osl = slice(128 * och, 128 * och + 128)
                    for dch in range(8):
                        nc.tensor.matmul(
                            ps[:], wt[dch][:, osl], xt[dch][:],
                            start=(dch == 0), stop=(dch == 7))
                    nc.scalar.activation(
                        dst[och][:], ps[:], AF.Identity,
                        bias=b_sb[:, och:och + 1], scale=1.0)

                def v_group(sub):
                    # both output halves share the stationary xt strip
                    vt = vts[sub]
                    ps = [proj_ps.tile([128, 512], F32, tag="proj",
                                       name=f"vps{sub}{oh}_{ti}")
                          for oh in range(2)]
                    xsl = slice(128 * sub, 128 * sub + 128)
                    for dch in range(8):
                        for oh in range(2):
                            nc.tensor.matmul(
                                ps[oh][:], xt[dch][:, xsl],
                                wvt[dch][:, 512 * oh:512 * oh + 512],
                                start=(dch == 0),
                                stop=(dch == 7) and not use_bv,
                                skip_group_check=True)
                    for oh in range(2):
                        sl = slice(512 * oh, 512 * oh + 512)
                        if use_bv:
                            nc.tensor.matmul(
                                ps[oh][:], ones_r[:], bv_r[:, sl],
                                start=False, stop=True,
                                skip_group_check=True)
                        nc.scalar.copy(vt[:, sl], ps[oh][:])

                for wt, dst, b_sb in ((wqt, qt, bq_sb), (wkt, kt, bk_sb)):
                    for och in range(8):
                        groups.append(
                            lambda wt=wt, dst=dst, b_sb=b_sb, och=och:
                            qk_group(wt, dst, b_sb, och))
                for sub in range(SUB):
                    groups.append(lambda sub=sub: v_group(sub))
                return qt, kt, vts, groups

            def make_att_rows(ti, qt, kt, vts, la=1):
                # Row-pair units: rows (2pi, 2pi+1) live on partition strips
                # (0, 64) of shared PSUM banks; all 16 heads of a row stay on
                # its strip, so v's natural layout feeds ctx directly (no
                # duplication) and the output DMA is fully contiguous.
                pairs = rows_per_tile // 2
                pairstate = {}

                def att_pair_a(pi):
                    pr = ti * pairs + pi          # global pair index

                    simt2 = rowp.tile([128, M], BF16, tag="sim2",
                                      name=f"sim2_{pr}")
                    nc.gpsimd.dma_start(simt2[0:64, :], sim_d[2 * pr, :, :])
                    nc.gpsimd.dma_start(simt2[64:128, :],
                                        sim_d[2 * pr + 1, :, :])

                    # scores transposed: S'[j, q]; bank bk holds heads of
                    # parity bk (h = 2hh+bk), contraction strip fixed at
                    # 64*bk, so each PE tile owns a unique (bank, half):
                    #   A[0:64]=T0  A[64:128]=T2  B[0:64]=T8  B[64:128]=T10
                    # (concurrent row-groups never share psum partitions)
                    tsl_e = slice(M * 2 * pi, M * 2 * pi + M)
                    tsl_o = slice(M * (2 * pi + 1), M * (2 * pi + 1) + M)
                    banks = [att_ps.tile([128, 512], F32, tag="att",
                                         name=f"s_{pr}_{bk}")
                             for bk in range(2)]
                    for hh in range(8):
                        for bk in range(2):
                            st = 64 * bk
                            for strip, tsl in ((0, tsl_e), (64, tsl_o)):
                                nc.tensor.matmul(
                                    banks[bk][strip:strip + 64,
                                              64 * hh:64 * hh + 64],
                                    kt[hh][st:st + 64, tsl],
                                    qt[hh][st:st + 64, tsl],
                                    start=True, stop=True,
                                    tile_position=(st, strip),
                                )

                    pts = []
                    for bk in range(2):
                        s_ps = banks[bk]
                        # t = S' * simT;  e = exp(t + maskcol)  (bf16 out)
                        tt = rowp.tile([128, 512], F32, tag=f"tt{bk}",
                                       name=f"tt_{pr}_{bk}")
                        nc.vector.tensor_tensor(
                            tt[:].rearrange("p (a j) -> p a j", j=M),
                            s_ps[:].rearrange("p (a j) -> p a j", j=M),
                            simt2[:].rearrange("p (a j) -> p a j", a=1)
                            .broadcast_to([128, 8, M]),
                            op=ALU.mult,
                        )
                        et = rowp.tile([128, 512], BF16, tag=f"et{bk}",
                                       name=f"et_{pr}_{bk}")
                        nc.scalar.activation(et[:], tt[:], AF.Exp,
                                             bias=mcol[:, pr:pr + 1])

                        # denominators summed + broadcast in one PE matmul
                        dn_ps = small_ps.tile([128, 512], F32, tag="srb",
                                              name=f"dn_{pr}_{bk}")
                        nc.tensor.matmul(dn_ps[:], halfones[:], et[:],
                                         start=True, stop=True)
                        nc.vector.reciprocal_approx_fast(out=dn_ps[:],
                                                         in_=dn_ps[:])
                        pt = rowp.tile([128, 512], BF16, tag=f"pt{bk}",
                                       name=f"pt_{pr}_{bk}")
                        nc.vector.tensor_tensor(pt[:], et[:], dn_ps[:],
                                                op=ALU.mult)
                        pts.append(pt)
                    pairstate[pi] = pts

                def att_pair_b(pi):
                    pr = ti * pairs + pi
                    pts = pairstate.pop(pi)
                    vt = vts[pi]   # [128 tok (both rows), DIM] natural

                    # ctx tile ownership (unique bank+half per PE tile):
                    #   ctxA[0:64]=T0(even)  ctxA[64:128]=T10(odd)
                    #   ctxB[0:64]=T8(odd)   ctxB[64:128]=T2(even)
                    ctxs = [att_ps.tile([128, 512], F32, tag="att",
                                        name=f"ctx_{pr}_{bk}")
                            for bk in range(2)]
                    for hh in range(8):
                        for bk in range(2):
                            h = 2 * hh + bk
                            for js, outs in (((0, 0) if bk == 0 else (0, 64)),
                                             ((64, 64) if bk == 0
                                              else (64, 0)),):
                                nc.tensor.matmul(
                                    ctxs[bk][outs:outs + 64,
                                             64 * hh:64 * hh + 64],
                                    pts[bk][js:js + 64,
                                            64 * hh:64 * hh + 64],
                                    vt[js:js + 64, 64 * h:64 * h + 64],
                                    start=True, stop=True,
                                    tile_position=(js, outs),
                                )
                    # osb_A: [0:64]=even row, [64:128]=odd row (even heads)
                    # osb_B: [0:64]=odd row,  [64:128]=even row (odd heads)
                    # out_dev columns parity-blocked; host permutes back
                    for bk in range(2):
                        osb = rowp.tile([128, 512], BF16, tag=f"osb{bk}",
                                        name=f"osb_{pr}_{bk}", bufs=2)
                        nc.scalar.copy(osb[:], ctxs[bk][:])
                        for sp in range(2):
                            r = 2 * pr + (sp if bk == 0 else 1 - sp)
                            nc.sync.dma_start(
                                out_d[M * r:M * r + M,
                                      512 * bk:512 * bk + 512],
                                osb[64 * sp:64 * sp + 64, :])

                # lookahead pipeline (la pairs of softmax chain in flight)
                units = [lambda pi=pi: att_pair_a(pi) for pi in range(la)]
                for pi in range(la, pairs):
                    units.append(lambda pi=pi: att_pair_a(pi))
                    units.append(lambda pi=pi: att_pair_b(pi - la))
                for pi in range(pairs - la, pairs):
                    units.append(lambda pi=pi: att_pair_b(pi))
                return units

            prev_rows = []
            for ti in range(n_tiles):
                xt = xt0 if ti == 0 else emit_xt(ti)
                qt, kt, vts, groups = make_proj(ti, xt)
                ri = 0
                for gi, g in enumerate(groups):
                    g()
                    while (ri < len(prev_rows)
                           and (gi + 1) * len(prev_rows) // len(groups) > ri):
                        prev_rows[ri]()
                        ri += 1
                while ri < len(prev_rows):
                    prev_rows[ri]()
                    ri += 1
                prev_rows = make_att_rows(
                    ti, qt, kt, vts, la=(2 if ti == n_tiles - 1 else 1))
            for row in prev_rows:
                row()

    return dict(out=out_d)


def _prepare_shards(hidden_states, attention_mask, sim_graph, Wq, bq, Wk, bk, Wv, bv,
                    n_cores=N_CORES):
    b, m, seq, dim = hidden_states.shape
    R = b * seq
    hs = np.transpose(np.asarray(hidden_states), (0, 2, 1, 3)).reshape(R, m, dim)
    import ml_dtypes
    bf16 = ml_dtypes.bfloat16
    am = np.ascontiguousarray(
        np.transpose(np.asarray(attention_mask), (0, 2, 1)).reshape(R, m),
        dtype=np.float32)
    sim = np.ascontiguousarray(
        np.transpose(np.asarray(sim_graph), (0, 2, 1)).astype(bf16))
    ident = np.eye(128, dtype=np.float32)
    selm = np.zeros((128, 2), np.float32)
    selm[0:64, 0] = 1.0
    selm[64:128, 1] = 1.0
    bselm = np.zeros((2, 128), np.float32)
    bselm[0, 0:64] = 1.0
    bselm[1, 64:128] = 1.0
    WqT = np.ascontiguousarray((np.asarray(Wq).T * 0.125).astype(bf16))
    WkT = np.ascontiguousarray(np.asarray(Wk).T.astype(bf16))
    WvT = np.ascontiguousarray(np.asarray(Wv).T.astype(bf16))
    rows_per_core = R // n_cores
    in_maps = []
    for c in range(n_cores):
        r0 = c * rows_per_core
        xT = np.ascontiguousarray(
            hs[r0:r0 + rows_per_core].reshape(rows_per_core * m, dim).T
            .astype(bf16))
        in_maps.append(dict(
            xT=xT,
            simg=sim[r0:r0 + rows_per_core],
            am=am[r0:r0 + rows_per_core],
            WqT=WqT, WkT=WkT, WvT=WvT,
            bq=np.ascontiguousarray(np.asarray(bq) * 0.125, np.float32),
            bk=np.ascontiguousarray(bk, np.float32),
            bv=np.ascontiguousarray(bv, np.float32),
            ident=ident, selm=selm, bselm=bselm,
        ))
    return in_maps


_CACHE = {}


def _get_compiled(use_bv=True):
    key = ("nc", use_bv)
    if key not in _CACHE:
        nc = bacc.Bacc("TRN2", target_bir_lowering=False, debug=False)
        build_core_kernel(nc, use_bv=use_bv)
        nc.compile()
        _CACHE[key] = nc
    return _CACHE[key]


LAST_EXEC_NS = [None]
LAST_RESULT = [None]


def kernel(hidden_states, attention_mask, sim_graph, Wq, bq, Wk, bk, Wv, bv,
           b=4, m=64, seq=256, dim=1024, **_):
    import os
    from concourse.bass_utils import run_bass_kernel_spmd

    use_bv = bool(np.any(np.asarray(bv)))
    nc = _get_compiled(use_bv=use_bv)
    in_maps = _prepare_shards(hidden_states, attention_mask, sim_graph,
                              Wq, bq, Wk, bk, Wv, bv)
    trace = bool(int(os.environ.get("BERT_TRACE", "0")))
    if trace:
        try:  # register the NTFF hook if the middleware didn't
            from antenv.axon_hooks import (get_axon_ntff_profile_hook,
                                           set_axon_ntff_profile_hook)
            if get_axon_ntff_profile_hook() is None:
                from trn_agent_boot.trn_boot import _ntff_profile_via_ctypes
                set_axon_ntff_profile_hook(
                    _ntff_profile_via_ctypes("/opt/axon/libaxon_pjrt.so"))
        except Exception:
            trace = False
    res = run_bass_kernel_spmd(nc, in_maps, list(range(N_CORES)), trace=trace)
    LAST_EXEC_NS[0] = res.exec_time_ns
    LAST_RESULT[0] = res
    R = int(b) * int(seq)
    out = np.concatenate([np.asarray(res.results[c]["out"], dtype=np.float32)
                          for c in range(N_CORES)], axis=0)
    # device column layout is parity-blocked: col = 512*(h%2) + 64*(h//2) + hd
    out = out.reshape(-1, 2, 8, 64).transpose(0, 2, 1, 3)
    return np.ascontiguousarray(out).reshape(R, int(m), int(dim))



# revision 31
# speedup vs baseline: 1.0483x; 1.0204x over previous
"""Trainium2 Bass kernel for BertSimSelfAttention (sparse_attention).

Problem (full): B=4, M=64, SEQ=256, DIM=1024, H=16, HD=64.
Effective batch rows R = B*SEQ = 1024, each row: m=64 tokens of dim=1024.
  hs  = transpose(hidden_states,(0,2,1,3)).reshape(R, 64, 1024)
  q/k/v = hs @ W{q,k,v}.T + b   (per token)
  per (row, head): scores = (q @ k.T)/8 * sim[row] + (-1e4)*(1-am[row,j])
  probs = softmax_j(scores);  ctx = probs @ v  -> out [R, 64, 1024]

Sharding: data-parallel over rows, 128 rows/core x 8 cores. The host
pre-transposes x, W (and sim per row) and converts them to bf16 so the
device consumes contraction-major bf16 layouts directly; the 1/sqrt(hd)
scale is folded into Wq/bq on the host.

Per-core design (~0.84 ms traced / ~0.70 ms untraced on TRN2):
  - Everything matmul-side is bf16 (fp32 PSUM accumulate). bf16
    128-col LDWEIGHTS (~107ns, FWL) hides under the 512-col moving
    stream (213ns); fp32r stationaries would gate the pipe at ~273ns.
  - Projections: one 512-col stream per PSUM bank (one FWL load per
    stream window -- interleaving two streams saturates the weight
    path and regresses). q/k bias added by ACT Identity at evacuation;
    v bias accumulated as a K=1 matmul (skipped when bv == 0).
  - Attention runs per ROW-PAIR on 64x64 PE tiles with every tile
    owning a unique (PSUM bank, partition half) -- concurrent PE row
    groups must never write the same PSUM partitions:
      scores  S'[j,q]: bank A (even heads, contraction strip 0)
        T0=even row [0:64], T2=odd row [64:128]; bank B (odd heads,
        strip 64): T8=even [0:64], T10=odd [64:128].
      ctx: T0=ctxA[0:64] (even), T10=ctxA[64:128] (odd),
        T8=ctxB[0:64] (odd), T2=ctxB[64:128] (even).
    The 4-way tile spread runs scores/ctx matmuls concurrently
    (~15ns/MM vs 27ns serial).
  - softmax over j (= partitions): t = S' * simT (DVE, sim bf16 via
    stride-0 broadcast); e = exp(t + maskcol) on ACT with a
    per-partition row-pair bias column (masked lanes give exact 0);
    denominators via one PE matmul per bank with a block-ones
    stationary; reciprocal_approx_fast in-place on PSUM;
    probs.T = e * recip (bf16) feeds ctx directly as stationary.
  - v's natural [t, o] layout feeds ctx moving operands with no
    duplication; output DMAs are contiguous [64, 512] bf16 lines in a
    parity-blocked column layout that the host permutes back.
  - Emission software-pipelines tile i's 12 projection groups with
    tile (i-1)'s attention pair units (one-pair lookahead).
"""

import sys

sys.path.insert(0, "/opt/trn_rl_repo")

import numpy as np
import concourse.bass as bass
import concourse.bacc as bacc
import concourse.mybir as mybir
import concourse.tile as tile

F32 = mybir.dt.float32
F32R = mybir.dt.float32r
BF16 = mybir.dt.bfloat16
AF = mybir.ActivationFunctionType
ALU = mybir.AluOpType

N_CORES = 8
M = 64                    # tokens per row
DIM = 1024
H = 16
HD = 64
NEG = -10000.0


def build_core_kernel(nc, n_tiles=16, rows_per_tile=8, debug=False, use_bv=True):
    """Emit the per-core program. tile = rows_per_tile rows (must be even)."""
    T_TILE = rows_per_tile * M        # tokens per tile (512 default)
    n_rows = n_tiles * rows_per_tile
    n_tok = n_rows * M
    SUB = T_TILE // 128               # 128-token subtiles per tile

    xt_d = nc.dram_tensor("xT", (DIM, n_tok), BF16, kind="ExternalInput")
    sim_d = nc.dram_tensor("simg", (n_rows, M, M), BF16, kind="ExternalInput")
    am_d = nc.dram_tensor("am", (n_rows, M), F32, kind="ExternalInput")
    wq_d = nc.dram_tensor("WqT", (DIM, DIM), BF16, kind="ExternalInput")
    wk_d = nc.dram_tensor("WkT", (DIM, DIM), BF16, kind="ExternalInput")
    wv_d = nc.dram_tensor("WvT", (DIM, DIM), BF16, kind="ExternalInput")
    bq_d = nc.dram_tensor("bq", (DIM,), F32, kind="ExternalInput")
    bk_d = nc.dram_tensor("bk", (DIM,), F32, kind="ExternalInput")
    bv_d = nc.dram_tensor("bv", (DIM,), F32, kind="ExternalInput")
    id_d = nc.dram_tensor("ident", (128, 128), F32, kind="ExternalInput")
    sel_d = nc.dram_tensor("selm", (128, 2), F32, kind="ExternalInput")
    bsel_d = nc.dram_tensor("bselm", (2, 128), F32, kind="ExternalInput")
    out_d = nc.dram_tensor("out", (n_tok, DIM), BF16, kind="ExternalOutput")

    dbg = {}
    if debug:
        dbg["qt"] = nc.dram_tensor("dbg_qt", (DIM, n_tok), F32, kind="ExternalOutput")
        dbg["kt"] = nc.dram_tensor("dbg_kt", (DIM, n_tok), F32, kind="ExternalOutput")
        dbg["v"] = nc.dram_tensor("dbg_v", (n_tok, DIM), F32, kind="ExternalOutput")
        dbg["pr"] = nc.dram_tensor("dbg_pr", (n_rows, 128, 512), F32,
                                   kind="ExternalOutput")
        dbg["s"] = nc.dram_tensor("dbg_s", (n_rows, 128, 512), F32,
                                  kind="ExternalOutput")

    with tile.TileContext(nc) as tc:
        with (
            tc.tile_pool(name="consts", bufs=1) as consts,
            tc.tile_pool(name="stage", bufs=3) as stage,
            tc.tile_pool(name="xtp", bufs=2) as xtp,
            tc.tile_pool(name="qkp", bufs=2) as qkp,
            tc.tile_pool(name="vp", bufs=2) as vp,
            tc.tile_pool(name="rowp", bufs=2) as rowp,
            tc.tile_pool(name="small_ps", bufs=2, space="PSUM") as small_ps,
            tc.tile_pool(name="proj_ps", bufs=2, space="PSUM") as proj_ps,
            tc.tile_pool(name="att_ps", bufs=4, space="PSUM") as att_ps,
        ):
            # ---------------- tiny consts first ----------------
            ident = consts.tile([128, 128], F32)
            nc.sync.dma_start(ident[:], id_d[:])

            am_all = consts.tile([128, M], F32)
            if n_rows < 128:
                nc.gpsimd.memset(am_all[:], 1.0)
            nc.sync.dma_start(am_all[0:n_rows, :], am_d[:])

            # block-ones selector: halfones[p, m] = 1 iff same 64-half;
            # halfones.T @ e sums each half and broadcasts to its partitions
            halfones = consts.tile([128, 128], BF16)
            nc.gpsimd.memset(halfones[:], 0.0)
            nc.gpsimd.memset(halfones[0:64, 0:64], 1.0)
            nc.gpsimd.memset(halfones[64:128, 64:128], 1.0)

            bq_sb = consts.tile([128, 8], F32)
            bk_sb = consts.tile([128, 8], F32)
            nc.sync.dma_start(bq_sb[:], bq_d[:].rearrange("(o p) -> p o", p=128))
            nc.sync.dma_start(bk_sb[:], bk_d[:].rearrange("(o p) -> p o", p=128))

            if use_bv:
                # bv as a K=1 bf16 pair for psum-accumulate
                ones_f = consts.tile([1, 128], F32)
                nc.gpsimd.memset(ones_f[:], 1.0)
                ones_r = consts.tile([1, 128], BF16)
                nc.vector.tensor_copy(ones_r[:], ones_f[:])
                bv_row = consts.tile([1, DIM], F32)
                nc.sync.dma_start(bv_row[:],
                                  bv_d[:].rearrange("(a o) -> a o", a=1))
                bv_r = consts.tile([1, DIM], BF16)
                nc.vector.tensor_copy(bv_r[:], bv_row[:])

            # mask bias columns, row-pair layout: mcol[64*(r%2)+j, r//2] =
            # -1e4*(1 - am[r, j])  (exp-bias per key token j, even row on
            # strip 0, odd row on strip 1)
            n_pairs = n_rows // 2
            mcol = consts.tile([128, n_pairs], F32)
            modd = consts.tile([64, n_pairs], F32)
            amt_ps = small_ps.tile([128, 128], F32, tag="srb")
            nc.tensor.transpose(amt_ps[0:M, 0:128], am_all[:], ident[:])
            amt_v = amt_ps[0:M, 0:2 * n_pairs].rearrange(
                "j (p two) -> j p two", two=2)
            nc.vector.tensor_scalar(
                mcol[0:64, :].rearrange("j (p a) -> j p a", a=1),
                amt_v[:, :, 0:1], -NEG, NEG,
                op0=ALU.mult, op1=ALU.add)
            nc.vector.tensor_scalar(
                modd[:].rearrange("j (p a) -> j p a", a=1),
                amt_v[:, :, 1:2], -NEG, NEG,
                op0=ALU.mult, op1=ALU.add)
            nc.gpsimd.dma_start(mcol[64:128, :], modd[:])

            # ---------------- weights (+ tile-0 x interleaved) ----------
            def emit_xt(ti):
                t0 = ti * T_TILE
                xt = [xtp.tile([128, T_TILE], BF16, tag=f"xt{d}",
                               name=f"xt{d}_{ti}") for d in range(8)]
                for dch in range(8):
                    nc.sync.dma_start(
                        xt[dch][:],
                        xt_d[128 * dch:128 * dch + 128, t0:t0 + T_TILE])
                return xt

            def emit_w(name, w_d, dchs):
                wt = wts[name]
                for dch in dchs:
                    weng = nc.gpsimd if dch % 2 else nc.sync
                    weng.dma_start(
                        wt[dch][:], w_d[128 * dch:128 * dch + 128, :])

            wts = {name: [consts.tile([128, DIM], BF16, tag=f"w{name}{d}",
                                      name=f"w{name}{d}") for d in range(8)]
                   for name in ("q", "k", "v")}
            xt0 = [xtp.tile([128, T_TILE], BF16, tag=f"xt{d}",
                             name=f"xt{d}_0") for d in range(8)]
            for dch in range(8):
                emit_w("q", wq_d, [dch])
                nc.sync.dma_start(xt0[dch][:],
                                  xt_d[128 * dch:128 * dch + 128, 0:T_TILE])
            emit_w("k", wk_d, range(8))
            emit_w("v", wv_d, range(8))
            wqt, wkt, wvt = wts["q"], wts["k"], wts["v"]

            # ---------------- main loop over token tiles ----------------
            # Emission interleaves tile ti's projection groups with tile
            # (ti-1)'s attention rows so the PE program order has dense
            # matmul work to fill softmax dependency stalls (keeps HAM warm).

            def make_proj(ti, xt):
                qt = [qkp.tile([128, T_TILE], BF16, tag=f"qt{o}",
                               name=f"qt{o}_{ti}") for o in range(8)]
                kt = [qkp.tile([128, T_TILE], BF16, tag=f"kt{o}",
                               name=f"kt{o}_{ti}") for o in range(8)]
                vts = [vp.tile([128, DIM], BF16, tag=f"v{s}",
                               name=f"v{s}_{ti}") for s in range(SUB)]
                groups = []

                def qk_group(wt, dst, b_sb, och):
                    # one bank, one FWL weight-load per 512-col stream --
                    # two interleaved streams saturate the LD path (XBUS)
                    ps = proj_ps.tile([128, T_TILE], F32, tag="proj",
                                      name=f"qkps{och}_{ti}")
                    osl = slice(128 * och, 128 * och + 128)
                    for dch in range(8):
                        nc.tensor.matmul(
                            ps[:], wt[dch][:, osl], xt[dch][:],
                            start=(dch == 0), stop=(dch == 7))
                    nc.scalar.activation(
                        dst[och][:], ps[:], AF.Identity,
                        bias=b_sb[:, och:och + 1], scale=1.0)

                def v_group(sub, oh):
                    vt = vts[sub]
                    ps = proj_ps.tile([128, 512], F32, tag="proj",
                                      name=f"vps{sub}{oh}_{ti}")
                    sl = slice(512 * oh, 512 * oh + 512)
                    for dch in range(8):
                        nc.tensor.matmul(
                            ps[:],
                            xt[dch][:, 128 * sub:128 * sub + 128],
                            wvt[dch][:, 512 * oh:512 * oh + 512],
                            start=(dch == 0), stop=(dch == 7) and not use_bv,
                        )
                    if use_bv:
                        nc.tensor.matmul(
                            ps[:], ones_r[:], bv_r[:, sl],
                            start=False, stop=True,
                        )
                    nc.scalar.copy(vt[:, sl], ps[:])

                for wt, dst, b_sb in ((wqt, qt, bq_sb), (wkt, kt, bk_sb)):
                    for och in range(8):
                        groups.append(
                            lambda wt=wt, dst=dst, b_sb=b_sb, och=och:
                            qk_group(wt, dst, b_sb, och))
                for sub in range(SUB):
                    for oh in range(2):
                        groups.append(lambda sub=sub, oh=oh: v_group(sub, oh))
                return qt, kt, vts, groups

            def make_att_rows(ti, qt, kt, vts, la=1):
                # Row-pair units: rows (2pi, 2pi+1) live on partition strips
                # (0, 64) of shared PSUM banks; all 16 heads of a row stay on
                # its strip, so v's natural layout feeds ctx directly (no
                # duplication) and the output DMA is fully contiguous.
                pairs = rows_per_tile // 2
                pairstate = {}

                def att_pair_a(pi):
                    pr = ti * pairs + pi          # global pair index

                    simt2 = rowp.tile([128, M], BF16, tag="sim2",
                                      name=f"sim2_{pr}")
                    nc.gpsimd.dma_start(simt2[0:64, :], sim_d[2 * pr, :, :])
                    nc.gpsimd.dma_start(simt2[64:128, :],
                                        sim_d[2 * pr + 1, :, :])

                    # scores transposed: S'[j, q]; bank bk holds heads of
                    # parity bk (h = 2hh+bk), contraction strip fixed at
                    # 64*bk, so each PE tile owns a unique (bank, half):
                    #   A[0:64]=T0  A[64:128]=T2  B[0:64]=T8  B[64:128]=T10
                    # (concurrent row-groups never share psum partitions)
                    tsl_e = slice(M * 2 * pi, M * 2 * pi + M)
                    tsl_o = slice(M * (2 * pi + 1), M * (2 * pi + 1) + M)
                    banks = [att_ps.tile([128, 512], F32, tag="att",
                                         name=f"s_{pr}_{bk}")
                             for bk in range(2)]
                    for hh in range(8):
                        for bk in range(2):
                            st = 64 * bk
                            for strip, tsl in ((0, tsl_e), (64, tsl_o)):
                                nc.tensor.matmul(
                                    banks[bk][strip:strip + 64,
                                              64 * hh:64 * hh + 64],
                                    kt[hh][st:st + 64, tsl],
                                    qt[hh][st:st + 64, tsl],
                                    start=True, stop=True,
                                    tile_position=(st, strip),
                                )

                    pts = []
                    for bk in range(2):
                        s_ps = banks[bk]
                        # t = S' * simT;  e = exp(t + maskcol)  (bf16 out)
                        tt = rowp.tile([128, 512], F32, tag=f"tt{bk}",
                                       name=f"tt_{pr}_{bk}")
                        nc.vector.tensor_tensor(
                            tt[:].rearrange("p (a j) -> p a j", j=M),
                            s_ps[:].rearrange("p (a j) -> p a j", j=M),
                            simt2[:].rearrange("p (a j) -> p a j", a=1)
                            .broadcast_to([128, 8, M]),
                            op=ALU.mult,
                        )
                        et = rowp.tile([128, 512], BF16, tag=f"et{bk}",
                                       name=f"et_{pr}_{bk}")
                        nc.scalar.activation(et[:], tt[:], AF.Exp,
                                             bias=mcol[:, pr:pr + 1])

                        # denominators summed + broadcast in one PE matmul
                        dn_ps = small_ps.tile([128, 512], F32, tag="srb",
                                              name=f"dn_{pr}_{bk}")
                        nc.tensor.matmul(dn_ps[:], halfones[:], et[:],
                                         start=True, stop=True)
                        nc.vector.reciprocal_approx_fast(out=dn_ps[:],
                                                         in_=dn_ps[:])
                        pt = rowp.tile([128, 512], BF16, tag=f"pt{bk}",
                                       name=f"pt_{pr}_{bk}")
                        nc.vector.tensor_tensor(pt[:], et[:], dn_ps[:],
                                                op=ALU.mult)
                        pts.append(pt)
                    pairstate[pi] = pts

                def att_pair_b(pi):
                    pr = ti * pairs + pi
                    pts = pairstate.pop(pi)
                    vt = vts[pi]   # [128 tok (both rows), DIM] natural

                    # ctx tile ownership (unique bank+half per PE tile):
                    #   ctxA[0:64]=T0(even)  ctxA[64:128]=T10(odd)
                    #   ctxB[0:64]=T8(odd)   ctxB[64:128]=T2(even)
                    ctxs = [att_ps.tile([128, 512], F32, tag="att",
                                        name=f"ctx_{pr}_{bk}")
                            for bk in range(2)]
                    for hh in range(8):
                        for bk in range(2):
                            h = 2 * hh + bk
                            for js, outs in (((0, 0) if bk == 0 else (0, 64)),
                                             ((64, 64) if bk == 0
                                              else (64, 0)),):
                                nc.tensor.matmul(
                                    ctxs[bk][outs:outs + 64,
                                             64 * hh:64 * hh + 64],
                                    pts[bk][js:js + 64,
                                            64 * hh:64 * hh + 64],
                                    vt[js:js + 64, 64 * h:64 * h + 64],
                                    start=True, stop=True,
                                    tile_position=(js, outs),
                                )
                    # osb_A: [0:64]=even row, [64:128]=odd row (even heads)
                    # osb_B: [0:64]=odd row,  [64:128]=even row (odd heads)
                    # out_dev columns parity-blocked; host permutes back
                    for bk in range(2):
                        osb = rowp.tile([128, 512], BF16, tag=f"osb{bk}",
                                        name=f"osb_{pr}_{bk}", bufs=2)
                        nc.scalar.copy(osb[:], ctxs[bk][:])
                        for sp in range(2):
                            r = 2 * pr + (sp if bk == 0 else 1 - sp)
                            nc.sync.dma_start(
                                out_d[M * r:M * r + M,
                                      512 * bk:512 * bk + 512],
                                osb[64 * sp:64 * sp + 64, :])

                # lookahead pipeline (la pairs of softmax chain in flight)
                units = [lambda pi=pi: att_pair_a(pi) for pi in range(la)]
                for pi in range(la, pairs):
                    units.append(lambda pi=pi: att_pair_a(pi))
                    units.append(lambda pi=pi: att_pair_b(pi - la))
                for pi in range(pairs - la, pairs):
                    units.append(lambda pi=pi: att_pair_b(pi))
                return units

            prev_rows = []
            for ti in range(n_tiles):
                xt = xt0 if ti == 0 else emit_xt(ti)
                qt, kt, vts, groups = make_proj(ti, xt)
                ri = 0
                for gi, g in enumerate(groups):
                    g()
                    while (ri < len(prev_rows)
                           and (gi + 1) * len(prev_rows) // len(groups) > ri):
                        prev_rows[ri]()
                        ri += 1
                while ri < len(prev_rows):
                    prev_rows[ri]()
                    ri += 1
                prev_rows = make_att_rows(
                    ti, qt, kt, vts, la=(2 if ti == n_tiles - 1 else 1))
            for row in prev_rows:
                row()

    return dict(out=out_d)


def _prepare_shards(hidden_states, attention_mask, sim_graph, Wq, bq, Wk, bk, Wv, bv,
                    n_cores=N_CORES):
    b, m, seq, dim = hidden_states.shape
    R = b * seq
    hs = np.transpose(np.asarray(hidden_states), (0, 2, 1, 3)).reshape(R, m, dim)
    import ml_dtypes
    bf16 = ml_dtypes.bfloat16
    am = np.ascontiguousarray(
        np.transpose(np.asarray(attention_mask), (0, 2, 1)).reshape(R, m),
        dtype=np.float32)
    sim = np.ascontiguousarray(
        np.transpose(np.asarray(sim_graph), (0, 2, 1)).astype(bf16))
    ident = np.eye(128, dtype=np.float32)
    selm = np.zeros((128, 2), np.float32)
    selm[0:64, 0] = 1.0
    selm[64:128, 1] = 1.0
    bselm = np.zeros((2, 128), np.float32)
    bselm[0, 0:64] = 1.0
    bselm[1, 64:128] = 1.0
    WqT = np.ascontiguousarray((np.asarray(Wq).T * 0.125).astype(bf16))
    WkT = np.ascontiguousarray(np.asarray(Wk).T.astype(bf16))
    WvT = np.ascontiguousarray(np.asarray(Wv).T.astype(bf16))
    rows_per_core = R // n_cores
    in_maps = []
    for c in range(n_cores):
        r0 = c * rows_per_core
        xT = np.ascontiguousarray(
            hs[r0:r0 + rows_per_core].reshape(rows_per_core * m, dim).T
            .astype(bf16))
        in_maps.append(dict(
            xT=xT,
            simg=sim[r0:r0 + rows_per_core],
            am=am[r0:r0 + rows_per_core],
            WqT=WqT, WkT=WkT, WvT=WvT,
            bq=np.ascontiguousarray(np.asarray(bq) * 0.125, np.float32),
            bk=np.ascontiguousarray(bk, np.float32),
            bv=np.ascontiguousarray(bv, np.float32),
            ident=ident, selm=selm, bselm=bselm,
        ))
    return in_maps


_CACHE = {}


def _get_compiled(use_bv=True):
    key = ("nc", use_bv)
    if key not in _CACHE:
        nc = bacc.Bacc("TRN2", target_bir_lowering=False, debug=False)
        build_core_kernel(nc, use_bv=use_bv)
        nc.compile()
        _CACHE[key] = nc
    return _CACHE[key]


LAST_EXEC_NS = [None]
LAST_RESULT = [None]


def kernel(hidden_states, attention_mask, sim_graph, Wq, bq, Wk, bk, Wv, bv,
           b=4, m=64, seq=256, dim=1024, **_):
    import os
    from concourse.bass_utils import run_bass_kernel_spmd

    use_bv = bool(np.any(np.asarray(bv)))
    nc = _get_compiled(use_bv=use_bv)
    in_maps = _prepare_shards(hidden_states, attention_mask, sim_graph,
                              Wq, bq, Wk, bk, Wv, bv)
    trace = bool(int(os.environ.get("BERT_TRACE", "0")))
    if trace:
        try:  # register the NTFF hook if the middleware didn't
            from antenv.axon_hooks import (get_axon_ntff_profile_hook,
                                           set_axon_ntff_profile_hook)
            if get_axon_ntff_profile_hook() is None:
                from trn_agent_boot.trn_boot import _ntff_profile_via_ctypes
                set_axon_ntff_profile_hook(
                    _ntff_profile_via_ctypes("/opt/axon/libaxon_pjrt.so"))
        except Exception:
            trace = False
    res = run_bass_kernel_spmd(nc, in_maps, list(range(N_CORES)), trace=trace)
    LAST_EXEC_NS[0] = res.exec_time_ns
    LAST_RESULT[0] = res
    R = int(b) * int(seq)
    out = np.concatenate([np.asarray(res.results[c]["out"], dtype=np.float32)
                          for c in range(N_CORES)], axis=0)
    # device column layout is parity-blocked: col = 512*(h%2) + 64*(h//2) + hd
    out = out.reshape(-1, 2, 8, 64).transpose(0, 2, 1, 3)
    return np.ascontiguousarray(out).reshape(R, int(m), int(dim))

